# revision 22
# baseline (speedup 1.0000x reference)
"""Trainium2 Bass kernel for nn_EncoderLayer (dense transformer layer).

Sharding: token-parallel. 8 cores; core c handles batch b=c//4, query slice
[q0, q0+512) with q0=(c%4)*512. The reference softmax is over the HEADS axis
(torch F.softmax implicit dim=1 on a 4D tensor), which couples all heads at a
given (q, k) but nothing across q -- so q-sharding needs no collectives. Each
core recomputes K/V for its whole batch element.

Numerics: projections/FFN1/out-proj in float32r (TF32-like, full PE rate),
attention QK^T / A@V and FFN2 in bf16. PSUM accumulation is always fp32.
Residual/bias adds are folded into PE accumulation via identity / ones-row
append-matmuls.
"""

import sys

sys.path.insert(0, "/opt/trn_rl_repo")

import numpy as np

import concourse.bass as bass
import concourse.mybir as mybir
import concourse.tile as tile
from concourse import bacc

f32 = mybir.dt.float32
f32r = mybir.dt.float32r
bf16 = mybir.dt.bfloat16
AF = mybir.ActivationFunctionType
ALU = mybir.AluOpType

D = 1024
S = 2048
H = 16
HD = 64
FF = 4096
NCORES = 8
TOK = 512           # tokens per core
QB = 256            # q block
NQB = TOK // QB     # 2
KT = 128            # k tile
NKT = S // KT       # 16
NP = H // 2         # 8 head pairs
EPS = 1e-5


def build(rep=1, dbg=False, phases=99):
    nc = bacc.Bacc("TRN2", target_bir_lowering=False, debug=False,
                   num_devices=NCORES)
    io = {}
    io["xT"] = nc.dram_tensor("xT", [D, S], bf16, kind="ExternalInput")
    io["xqT"] = nc.dram_tensor("xqT", [D, TOK], bf16, kind="ExternalInput")
    io["xs"] = nc.dram_tensor("xs", [TOK, D], f32r, kind="ExternalInput")
    io["wq"] = nc.dram_tensor("wq", [D, D], bf16, kind="ExternalInput")
    io["wk"] = nc.dram_tensor("wk", [D, D], bf16, kind="ExternalInput")
    io["wv"] = nc.dram_tensor("wv", [D, D], bf16, kind="ExternalInput")
    io["bqk"] = nc.dram_tensor("bqk", [128, 16], f32, kind="ExternalInput")
    io["bvr"] = nc.dram_tensor("bvr", [1, D], bf16, kind="ExternalInput")
    io["wo"] = nc.dram_tensor("wo", [D, D], f32r, kind="ExternalInput")
    io["w1"] = nc.dram_tensor("w1", [D, FF], bf16, kind="ExternalInput")
    io["b1p"] = nc.dram_tensor("b1p", [128, 32], f32, kind="ExternalInput")
    io["w2"] = nc.dram_tensor("w2", [2, 128, FF * 4], bf16, kind="ExternalInput")
    io["b2r"] = nc.dram_tensor("b2r", [1, D], bf16, kind="ExternalInput")
    io["g1"] = nc.dram_tensor("g1", [1, D], f32, kind="ExternalInput")
    io["be1"] = nc.dram_tensor("be1", [1, D], f32, kind="ExternalInput")
    io["g2"] = nc.dram_tensor("g2", [1, D], f32, kind="ExternalInput")
    io["be2"] = nc.dram_tensor("be2", [1, D], f32, kind="ExternalInput")
    io["identr"] = nc.dram_tensor("identr", [128, 128], f32r,
                                  kind="ExternalInput")
    io["ones1"] = nc.dram_tensor("ones1", [1, 128], bf16, kind="ExternalInput")
    io["y"] = nc.dram_tensor("y", [TOK, D], f32, kind="ExternalOutput")
    io["dbg"] = None
    if dbg:
        io["dbg"] = {
            "qT": nc.dram_tensor("dbg_qT", [NP, 128, TOK], bf16, kind="ExternalOutput"),
            "kT": nc.dram_tensor("dbg_kT", [NP, 128, S], bf16, kind="ExternalOutput"),
            "v": nc.dram_tensor("dbg_v", [NKT, 128, D], bf16, kind="ExternalOutput"),
            "A0": nc.dram_tensor("dbg_A0", [128, H, QB], bf16, kind="ExternalOutput"),
            "den0": nc.dram_tensor("dbg_den0", [128, QB], f32, kind="ExternalOutput"),
            "vT": nc.dram_tensor("dbg_vT", [NP, 128, QB], f32, kind="ExternalOutput"),
            "x0": nc.dram_tensor("dbg_x0", [2, 128, D], f32, kind="ExternalOutput"),
            "x1": nc.dram_tensor("dbg_x1", [2, 128, D], f32, kind="ExternalOutput"),
            "h1": nc.dram_tensor("dbg_h1", [4, 128, QB], bf16, kind="ExternalOutput"),
            "z": nc.dram_tensor("dbg_z", [2, 128, D], f32, kind="ExternalOutput"),
        }

    with tile.TileContext(nc) as tc:
        from contextlib import ExitStack
        with ExitStack() as ctx:
            _emit(nc, tc, rep, io, ctx, phases)
    nc.compile()
    return nc


def _bcast_mid(t, n):
    # [128, Q] AP -> [128, n, Q] with a zero-step middle dim (free re-read)
    a = t[:]
    return bass.AP(tensor=a.tensor, offset=a.offset,
                   ap=[a.ap[0], [0, n], a.ap[1]])


def _emit(nc, tc, rep, io, ctx, phases=99):
    const = ctx.enter_context(tc.tile_pool(name="const", bufs=1))
    C = {}
    C["identr"] = const.tile([128, 128], f32r, name="identr_t")
    C["ones1"] = const.tile([1, 128], bf16, name="ones1_t")
    C["bqk"] = const.tile([128, 16], f32, name="bqk_t")
    C["b1p"] = const.tile([128, 32], f32, name="b1p_t")
    C["bvr"] = const.tile([1, D], bf16, name="bvr_t")
    C["b2r"] = const.tile([1, D], bf16, name="b2r_t")
    for nm in ("identr", "ones1", "bqk", "b1p", "bvr", "b2r"):
        nc.sync.dma_start(out=C[nm][:], in_=io[nm][:])
    C["eps"] = const.tile([128, 1], f32, name="eps_t")
    nc.vector.memset(C["eps"], EPS)
    # replicated [128, D] bf16 row constants (DVE cannot partition-broadcast)
    for nm in ("g1", "be1", "g2", "be2"):
        t = const.tile([128, D], bf16, name=f"row_{nm}")
        a = io[nm][:]
        nc.gpsimd.dma_start(out=t[:], in_=bass.AP(
            tensor=a.tensor, offset=a.offset, ap=[[0, 128], a.ap[1]]))
        C[nm] = t

    for r in range(rep):
        from contextlib import ExitStack
        with ExitStack() as bctx:
            _emit_body(nc, tc, r, io, C, bctx, io.get("dbg") if r == 0 else None, phases)


def _emit_body(nc, tc, r, io, C, bctx, dbg=None, phases=99):
    p = f"r{r}_"
    xT, xqT, xs = io["xT"], io["xqT"], io["xs"]
    wq, wk, wv, wo, w1, w2, y = (io["wq"], io["wk"], io["wv"], io["wo"],
                                 io["w1"], io["w2"], io["y"])
    identr, ones1, bvr, b2r = C["identr"], C["ones1"], C["bvr"], C["b2r"]

    kvq = bctx.enter_context(tc.tile_pool(name=p + "kvq", bufs=1))
    kT_sb = [kvq.tile([128, S], bf16, name=f"{p}kT{i}") for i in range(NP)]
    qT_sb = [kvq.tile([128, TOK], bf16, name=f"{p}qT{i}") for i in range(NP)]
    v_sb = [kvq.tile([128, D], bf16, name=f"{p}v{i}") for i in range(NKT)]

    ps_mm = bctx.enter_context(tc.tile_pool(name=p + "ps_mm", bufs=2,
                                            space="PSUM"))
    ps_sc = bctx.enter_context(tc.tile_pool(name=p + "ps_sc", bufs=2,
                                            space="PSUM"))
    ps_av = bctx.enter_context(tc.tile_pool(name=p + "ps_av", bufs=1,
                                            space="PSUM"))
    xst = bctx.enter_context(tc.tile_pool(name=p + "xst", bufs=16))
    att = bctx.enter_context(tc.tile_pool(name=p + "att", bufs=1))
    work = bctx.enter_context(tc.tile_pool(name=p + "work", bufs=1))

    def wtile(nm, dt=f32r):
        return xst.tile([128, 512], dt, tag="xtc", name=nm)

    # ---- K^T projection: kT[pp] = [dk(2 heads x 64), tok] (bf16) ----
    wkh = [xst.tile([128, D], bf16, tag="xtc", name=f"{p}wkh{k}")
           for k in range(8)]
    for k in range(8):
        nc.scalar.dma_start(out=wkh[k][:], in_=wk[k * 128:(k + 1) * 128, :])
    for cc in range(2):
        xcb = [xst.tile([128, 1024], bf16, tag="xtc", name=f"{p}xcb{cc}_{k}")
               for k in range(8)]
        for k in range(8):
            nc.scalar.dma_start(out=xcb[k][:],
                                in_=xT[k * 128:(k + 1) * 128,
                                      cc * 1024:(cc + 1) * 1024])
        for h5 in range(2):
            c = cc * 2 + h5
            for pp in range(NP):
                ps = ps_mm.tile([128, 512], f32, tag="mm",
                                name=f"{p}kps{c}_{pp}")
                for k in range(8):
                    nc.tensor.matmul(ps[:], wkh[k][:, pp * 128:(pp + 1) * 128],
                                     xcb[k][:, h5 * 512:(h5 + 1) * 512],
                                     start=(k == 0), stop=(k == 7))
                nc.scalar.activation(kT_sb[pp][:, c * 512:(c + 1) * 512],
                                     ps[:], AF.Identity,
                                     bias=C["bqk"][:, 8 + pp:9 + pp],
                                     scale=1.0)

    # ---- Q^T projection (bf16) ----
    wqh = [xst.tile([128, D], bf16, tag="xtc", name=f"{p}wqh{k}")
           for k in range(8)]
    xqb = [xst.tile([128, TOK], bf16, tag="xtc", name=f"{p}xqb{k}")
           for k in range(8)]
    for k in range(8):
        nc.scalar.dma_start(out=wqh[k][:], in_=wq[k * 128:(k + 1) * 128, :])
        nc.scalar.dma_start(out=xqb[k][:], in_=xqT[k * 128:(k + 1) * 128, :])
    for pp in range(NP):
        ps = ps_mm.tile([128, TOK], f32, tag="mm", name=f"{p}qps{pp}")
        for k in range(8):
            nc.tensor.matmul(ps[:], wqh[k][:, pp * 128:(pp + 1) * 128],
                             xqb[k][:], start=(k == 0), stop=(k == 7))
        nc.scalar.activation(qT_sb[pp][:], ps[:], AF.Identity,
                             bias=C["bqk"][:, pp:pp + 1], scale=1.0)

    # ---- V projection (bf16): emitted per-kt inside attention(0) ----
    wvh = [xst.tile([128, D], bf16, tag="xtc", name=f"{p}wvh{k}")
           for k in range(8)]
    for k in range(8):
        nc.scalar.dma_start(out=wvh[k][:], in_=wv[k * 128:(k + 1) * 128, :])
    xvb_cur = [None]

    def v_emit(kt):
        cc, kq = kt // 8, kt % 8
        if kq == 0:
            xvb_cur[0] = [xst.tile([128, 1024], bf16, tag="xtc",
                                   name=f"{p}xvb{cc}_{k}") for k in range(8)]
            for k in range(8):
                nc.scalar.dma_start(out=xvb_cur[0][k][:],
                                    in_=xT[k * 128:(k + 1) * 128,
                                          cc * 1024:(cc + 1) * 1024])
        xvb = xvb_cur[0]
        for n in range(2):
            ps = ps_mm.tile([128, 512], f32, tag="mm", name=f"{p}vps{kt}_{n}")
            for k in range(8):
                nc.tensor.matmul(ps[:], xvb[k][:, kq * 128:(kq + 1) * 128],
                                 wvh[k][:, n * 512:(n + 1) * 512],
                                 start=(k == 0), stop=False)
            nc.tensor.matmul(ps[:], ones1[:], bvr[:, n * 512:(n + 1) * 512],
                             start=False, stop=True)
            nc.vector.tensor_copy(v_sb[kt][:, n * 512:(n + 1) * 512], ps[:])

    x1_t = {}     # (qb, t) -> [128, D] f32r (LN1 out)
    x1T = {}      # (qb, j) -> [128, QB] bf16 (transposed LN1 out)
    vT = {}       # (qb, pp) -> [128, QB] f32r

    av = {}
    for qb in range(NQB):
        av[qb] = [ps_av.tile([128, 512], f32, tag=f"av{j}",
                             name=f"{p}av{qb}_{j}") for j in range(4)]

    def attention(qb):
        # multiple col-tiled accumulation streams share an av bank; start=True
        # clears per-bank state and corrupts sibling streams (verified on HW).
        # Pre-zero via DVE and accumulate with start=False throughout.
        for j in range(4):
            nc.vector.memset(av[qb][j][:], 0.0)
        for kt in range(NKT):
            if qb == 0:
                v_emit(kt)
            expall = att.tile([128, H, QB], bf16, tag="expall",
                              name=f"{p}ea{qb}_{kt}", bufs=2)
            for pp in range(NP):
                scs = []
                for i in range(2):
                    sc = ps_sc.tile([128, QB], f32, tag="sc",
                                    name=f"{p}sc{qb}_{kt}_{pp}_{i}")
                    nc.tensor.matmul(
                        sc[:],
                        kT_sb[pp][i * 64:(i + 1) * 64,
                                  kt * 128:(kt + 1) * 128],
                        qT_sb[pp][i * 64:(i + 1) * 64,
                                  qb * QB:(qb + 1) * QB],
                        start=True, stop=True, tile_position=(i * 64, 0))
                    scs.append(sc)
                for i in range(2):
                    nc.scalar.activation(expall[:, 2 * pp + i, :], scs[i][:],
                                         AF.Exp, scale=0.125)
            d4a = att.tile([128, 4, QB], bf16, tag="d4a", bufs=2,
                           name=f"{p}d4a_{qb}_{kt}")
            nc.vector.tensor_add(d4a[:], expall[:, 0:4, :], expall[:, 4:8, :])
            d4b = att.tile([128, 4, QB], bf16, tag="d4b", bufs=2,
                           name=f"{p}d4b_{qb}_{kt}")
            nc.vector.tensor_add(d4b[:], expall[:, 8:12, :],
                                 expall[:, 12:16, :])
            d2 = att.tile([128, 4, QB], bf16, tag="d2", bufs=2,
                          name=f"{p}d2_{qb}_{kt}")
            nc.vector.tensor_add(d2[:], d4a[:], d4b[:])
            d1 = att.tile([128, 2, QB], bf16, tag="d1", bufs=2,
                          name=f"{p}d1_{qb}_{kt}")
            nc.vector.tensor_add(d1[:], d2[:, 0:2, :], d2[:, 2:4, :])
            den = att.tile([128, QB], f32, tag="den", bufs=2,
                           name=f"{p}dn{qb}_{kt}")
            nc.vector.tensor_add(den[:], d1[:, 0, :], d1[:, 1, :])
            rden = att.tile([128, QB], f32, tag="rden", bufs=2,
                            name=f"{p}rd{qb}_{kt}")
            nc.vector.reciprocal_approx_fast(out=rden[:], in_=den[:])
            rdenb = att.tile([128, QB], bf16, tag="rdenb", bufs=2,
                             name=f"{p}rb{qb}_{kt}")
            nc.vector.tensor_copy(rdenb[:], rden[:])
            # normalize in place: A = exp * (1/den), den broadcast over heads
            nc.vector.tensor_mul(expall[:, 0:10, :], expall[:, 0:10, :],
                                 _bcast_mid(rdenb, 10))
            nc.gpsimd.tensor_mul(expall[:, 10:16, :], expall[:, 10:16, :],
                                 _bcast_mid(rdenb, 6))
            if dbg is not None and qb == 0 and kt == 0:
                nc.sync.dma_start(out=dbg["A0"][:], in_=expall[:])
                nc.sync.dma_start(out=dbg["den0"][:], in_=den[:])
            for pp in range(NP):
                j, half = pp // 2, pp % 2
                for i in range(2):
                    hg = 2 * pp + i
                    nc.tensor.matmul(
                        av[qb][j][i * 64:(i + 1) * 64,
                                  half * QB:(half + 1) * QB],
                        v_sb[kt][:, hg * 64:(hg + 1) * 64],
                        expall[:, hg, :],
                        start=False, stop=(kt == NKT - 1),
                        tile_position=(0, i * 64))

    def vals(qb):
        for pp in range(NP):
            j, half = pp // 2, pp % 2
            t = work.tile([128, QB], f32r, tag=f"vT{pp}",
                          name=f"{p}vT{qb}_{pp}")
            nc.vector.tensor_copy(t[:], av[qb][j][:, half * QB:(half + 1) * QB])
            if dbg is not None and qb == 0:
                nc.sync.dma_start(out=dbg["vT"][pp], in_=t[:].bitcast(f32))
            vT[(qb, pp)] = t

    def layernorm(nm, x0, gname, bname, out_t):
        # x0: [128, D] f32 sbuf tile; normalizes in place then writes out_t
        stats = work.tile([128, 2, 6], f32, tag="lnst", name=nm + "_st",
                          bufs=2)
        for s in range(2):
            nc.vector.bn_stats(out=stats[:, s, :],
                               in_=x0[:, s * 512:(s + 1) * 512])
        mv = work.tile([128, 2], f32, tag="lnmv", name=nm + "_mv", bufs=2)
        nc.vector.bn_aggr(out=mv[:], in_=stats[:])
        lnv = work.tile([128, 1], f32, tag="lnv", name=nm + "_lv", bufs=2)
        nc.scalar.activation(lnv[:], mv[:, 1:2], AF.Ln, bias=C["eps"][:],
                             scale=1.0)
        rstd = work.tile([128, 1], f32, tag="lnr", name=nm + "_lr", bufs=2)
        nc.scalar.activation(rstd[:], lnv[:], AF.Exp, scale=-0.5)
        nc.vector.tensor_scalar(x0[:], x0[:], mv[:, 0:1], rstd[:],
                                op0=ALU.subtract, op1=ALU.mult)
        nc.vector.tensor_mul(x0[:], x0[:], C[gname][:])
        nc.vector.tensor_add(out_t[:], x0[:], C[bname][:])

    def proj_ln1(qb):
        x0s = {}
        for n in range(2):
            woc = [wtile(f"{p}wo{qb}_{n}_{k}") for k in range(8)]
            for k in range(8):
                nc.scalar.dma_start(out=woc[k][:],
                                  in_=wo[k * 128:(k + 1) * 128,
                                        n * 512:(n + 1) * 512])
            for t in range(2):
                if n == 0:
                    x0s[t] = work.tile([128, D], f32, tag="x0",
                                       name=f"{p}x0_{qb}_{t}", bufs=2)
                row = qb * QB + t * 128
                xst_t = wtile(f"{p}xsl{qb}_{n}_{t}")
                nc.sync.dma_start(out=xst_t[:],
                                  in_=xs[row:row + 128,
                                         n * 512:(n + 1) * 512])
                ps = ps_mm.tile([128, 512], f32, tag="mm",
                                name=f"{p}ops{qb}_{n}_{t}")
                for k in range(8):
                    nc.tensor.matmul(ps[:],
                                     vT[(qb, k)][:, t * 128:(t + 1) * 128],
                                     woc[k][:], start=(k == 0), stop=False)
                nc.tensor.matmul(ps[:], identr[:], xst_t[:],
                                 start=False, stop=True)
                nc.scalar.copy(x0s[t][:, n * 512:(n + 1) * 512], ps[:])
        if dbg is not None and qb == 0:
            for t in range(2):
                nc.sync.dma_start(out=dbg["x0"][t], in_=x0s[t][:])
        for t in range(2):
            x1 = work.tile([128, D], f32r, tag="x1", name=f"{p}x1_{qb}_{t}",
                           bufs=2)
            layernorm(f"{p}ln1_{qb}_{t}", x0s[t], "g1", "be1", x1)
            if dbg is not None and qb == 0:
                nc.sync.dma_start(out=dbg["x1"][t], in_=x1[:].bitcast(f32))
            x1_t[(qb, t)] = x1
            for j in range(8):
                tr = ps_mm.tile([128, 128], f32r, tag="mm",
                                name=f"{p}tr{qb}_{t}_{j}")
                nc.tensor.transpose(tr[:], x1[:, j * 128:(j + 1) * 128],
                                    identr[:])
                if t == 0:
                    x1T[(qb, j)] = work.tile([128, QB], bf16, tag=f"x1T{j}",
                                             name=f"{p}x1T{qb}_{j}")
                nc.vector.tensor_copy(x1T[(qb, j)][:, t * 128:(t + 1) * 128], tr[:])

    def ffn(qb):
        h1 = {}
        for mc in range(4):
            w1c = [xst.tile([128, 1024], bf16, tag="xtc",
                            name=f"{p}w1_{qb}_{mc}_{k}") for k in range(8)]
            for k in range(8):
                nc.scalar.dma_start(out=w1c[k][:],
                                  in_=w1[k * 128:(k + 1) * 128,
                                        mc * 1024:(mc + 1) * 1024])
            for mi in range(8):
                m = mc * 8 + mi
                ps = ps_mm.tile([128, QB], f32, tag="mm",
                                name=f"{p}f1ps{qb}_{m}")
                for k in range(8):
                    nc.tensor.matmul(ps[:],
                                     w1c[k][:, mi * 128:(mi + 1) * 128],
                                     x1T[(qb, k)][:],
                                     start=(k == 0), stop=(k == 7))
                ht = work.tile([128, QB], bf16, tag=f"h1_{m}",
                               name=f"{p}h1_{qb}_{m}")
                nc.scalar.activation(ht[:], ps[:], AF.Relu,
                                     bias=C["b1p"][:, m:m + 1], scale=1.0)
                if dbg is not None and qb == 0 and m < 4:
                    nc.sync.dma_start(out=dbg["h1"][m], in_=ht[:])
                h1[m] = ht
        for n in range(2):
            w2c = [xst.tile([128, 1024], bf16, tag="xtc",
                            name=f"{p}w2_{qb}_{n}_{g}") for g in range(16)]
            for g in range(16):
                nc.scalar.dma_start(out=w2c[g][:],
                                    in_=w2[n, :, g * 1024:(g + 1) * 1024])
            for t in range(2):
                if n == 0:
                    x0s_f = work.tile([128, D], f32, tag="x0",
                                      name=f"{p}z{qb}_{t}", bufs=2)
                    zs[t] = x0s_f
                ps = ps_mm.tile([128, 512], f32, tag="mm",
                                name=f"{p}f2ps{qb}_{n}_{t}")
                for m in range(32):
                    nc.tensor.matmul(ps[:], h1[m][:, t * 128:(t + 1) * 128],
                                     w2c[m // 2][:, (m % 2) * 512:
                                                 (m % 2 + 1) * 512],
                                     start=(m == 0), stop=False)
                nc.tensor.matmul(ps[:], identr[:],
                                 x1_t[(qb, t)][:, n * 512:(n + 1) * 512],
                                 start=False, stop=False)
                nc.tensor.matmul(ps[:], ones1[:],
                                 b2r[:, n * 512:(n + 1) * 512],
                                 start=False, stop=True)
                nc.scalar.copy(zs[t][:, n * 512:(n + 1) * 512], ps[:])
        if dbg is not None and qb == 0:
            for t in range(2):
                nc.sync.dma_start(out=dbg["z"][t], in_=zs[t][:])
        for t in range(2):
            yrow = qb * QB + t * 128
            layernorm(f"{p}ln2_{qb}_{t}", zs[t], "g2", "be2", zs[t])
            nc.sync.dma_start(out=y[yrow:yrow + 128, :], in_=zs[t][:])

    zs = {}
    steps = [lambda: attention(0), lambda: vals(0), lambda: proj_ln1(0),
             lambda: attention(1), lambda: ffn(0), lambda: vals(1),
             lambda: proj_ln1(1), lambda: ffn(1)]
    for i, s in enumerate(steps):
        if i + 1 <= phases:
            s()


_cached = None


def _get_program():
    global _cached
    if _cached is None:
        _cached = build(rep=1)
    return _cached


def _tile_w2(w2):
    import ml_dtypes
    # [FF, D] -> [2, 128, FF*4]: w2t[n, p, m*512+c] = w2[m*128+p, n*512+c]
    w4 = w2.reshape(32, 128, 2, 512)            # m, p, n, c
    out = np.ascontiguousarray(w4.transpose(2, 1, 0, 3).reshape(2, 128, FF * 4))
    return out.astype(ml_dtypes.bfloat16)


def prepare_inputs(x, w_qkv, b_qkv, w_o, b_o, gamma1, beta1, w1, b1, w2, b2,
                   gamma2, beta2):
    import ml_dtypes
    x = np.asarray(x, np.float32)
    w_qkv = np.asarray(w_qkv, np.float32)
    b_qkv = np.asarray(b_qkv, np.float32)
    wq3 = w_qkv.reshape(D, H, 3 * HD)
    bq3 = b_qkv.reshape(H, 3 * HD)
    bq_h = bq3[:, 0:HD].reshape(D)
    bk_h = bq3[:, HD:2 * HD].reshape(D)
    bqk = np.stack([bq_h.reshape(8, 128), bk_h.reshape(8, 128)],
                   axis=0).reshape(16, 128).T  # [128, 16]: cols 0-7 q, 8-15 k
    shared = {
        "wq": np.asarray(wq3[:, :, 0:HD].reshape(D, D), ml_dtypes.bfloat16),
        "wk": np.asarray(wq3[:, :, HD:2 * HD].reshape(D, D),
                         ml_dtypes.bfloat16),
        "wv": np.asarray(wq3[:, :, 2 * HD:3 * HD].reshape(D, D),
                         ml_dtypes.bfloat16),
        "bqk": np.ascontiguousarray(bqk, dtype=np.float32),
        "bvr": np.asarray(bq3[:, 2 * HD:3 * HD].reshape(1, D),
                          ml_dtypes.bfloat16),
        "wo": np.asarray(w_o, np.float32),
        "w1": np.asarray(w1, ml_dtypes.bfloat16),
        "b1p": np.ascontiguousarray(
            np.asarray(b1, np.float32).reshape(32, 128).T),
        "w2": _tile_w2(np.asarray(w2, np.float32)),
        "b2r": np.asarray(np.asarray(b2, np.float32).reshape(1, D),
                          ml_dtypes.bfloat16),
        "g1": np.asarray(gamma1, np.float32).reshape(1, D),
        "be1": np.asarray(beta1, np.float32).reshape(1, D),
        "g2": np.asarray(gamma2, np.float32).reshape(1, D),
        "be2": np.asarray(beta2, np.float32).reshape(1, D),
        "identr": np.eye(128, dtype=np.float32),
        "ones1": np.ones((1, 128), ml_dtypes.bfloat16),
    }
    bo = np.asarray(b_o, np.float32)
    in_maps = []
    for c in range(NCORES):
        b, q0 = c // 4, (c % 4) * TOK
        xTb = np.ascontiguousarray(x[b].T)
        m = dict(shared)
        m["xT"] = np.asarray(xTb, ml_dtypes.bfloat16)
        m["xqT"] = np.asarray(xTb[:, q0:q0 + TOK], ml_dtypes.bfloat16)
        m["xs"] = np.ascontiguousarray(x[b, q0:q0 + TOK] + bo[None, :])
        in_maps.append(m)
    return in_maps


def kernel(**inputs):
    from concourse.bass_utils import run_bass_kernel_spmd
    nc = _get_program()
    in_maps = prepare_inputs(**inputs)
    res = run_bass_kernel_spmd(nc, in_maps, list(range(NCORES)))
    out = np.empty((2, S, D), np.float32)
    for c in range(NCORES):
        b, q0 = c // 4, (c % 4) * TOK
        out[b, q0:q0 + TOK] = res.results[c]["y"]
    return out


# revision 24
# speedup vs baseline: 1.0905x; 1.0905x over previous
"""Trainium2 Bass kernel for nn_EncoderLayer (dense transformer layer).

Sharding: token-parallel. 8 cores; core c handles batch b=c//4, query slice
[q0, q0+512) with q0=(c%4)*512. The reference softmax is over the HEADS axis
(torch F.softmax implicit dim=1 on a 4D tensor), which couples all heads at a
given (q, k) but nothing across q -- so q-sharding needs no collectives. Each
core recomputes K/V for its whole batch element.

Numerics: projections/FFN1/out-proj in float32r (TF32-like, full PE rate),
attention QK^T / A@V and FFN2 in bf16. PSUM accumulation is always fp32.
Residual/bias adds are folded into PE accumulation via identity / ones-row
append-matmuls.
"""

import sys

sys.path.insert(0, "/opt/trn_rl_repo")

import numpy as np

import concourse.bass as bass
import concourse.mybir as mybir
import concourse.tile as tile
from concourse import bacc

f32 = mybir.dt.float32
f32r = mybir.dt.float32r
bf16 = mybir.dt.bfloat16
AF = mybir.ActivationFunctionType
ALU = mybir.AluOpType

D = 1024
S = 2048
H = 16
HD = 64
FF = 4096
NCORES = 8
TOK = 512           # tokens per core
QB = 256            # q block
NQB = TOK // QB     # 2
KT = 128            # k tile
NKT = S // KT       # 16
NP = H // 2         # 8 head pairs
EPS = 1e-5


def build(rep=1, dbg=False, phases=99):
    nc = bacc.Bacc("TRN2", target_bir_lowering=False, debug=False,
                   num_devices=NCORES)
    io = {}
    io["xT"] = nc.dram_tensor("xT", [D, S], bf16, kind="ExternalInput")
    io["xqT"] = nc.dram_tensor("xqT", [D, TOK], bf16, kind="ExternalInput")
    io["xs"] = nc.dram_tensor("xs", [TOK, D], f32r, kind="ExternalInput")
    io["wq"] = nc.dram_tensor("wq", [D, D], bf16, kind="ExternalInput")
    io["wk"] = nc.dram_tensor("wk", [D, D], bf16, kind="ExternalInput")
    io["wv"] = nc.dram_tensor("wv", [D, D], bf16, kind="ExternalInput")
    io["bqk"] = nc.dram_tensor("bqk", [128, 16], f32, kind="ExternalInput")
    io["bvr"] = nc.dram_tensor("bvr", [1, D], bf16, kind="ExternalInput")
    io["wo"] = nc.dram_tensor("wo", [D, D], f32r, kind="ExternalInput")
    io["w1"] = nc.dram_tensor("w1", [D, FF], bf16, kind="ExternalInput")
    io["b1p"] = nc.dram_tensor("b1p", [128, 32], f32, kind="ExternalInput")
    io["w2"] = nc.dram_tensor("w2", [2, 128, FF * 4], bf16, kind="ExternalInput")
    io["b2r"] = nc.dram_tensor("b2r", [1, D], bf16, kind="ExternalInput")
    io["g1"] = nc.dram_tensor("g1", [1, D], f32, kind="ExternalInput")
    io["be1"] = nc.dram_tensor("be1", [1, D], f32, kind="ExternalInput")
    io["g2"] = nc.dram_tensor("g2", [1, D], f32, kind="ExternalInput")
    io["be2"] = nc.dram_tensor("be2", [1, D], f32, kind="ExternalInput")
    io["identr"] = nc.dram_tensor("identr", [128, 128], f32r,
                                  kind="ExternalInput")
    io["ones1"] = nc.dram_tensor("ones1", [1, 128], bf16, kind="ExternalInput")
    io["y"] = nc.dram_tensor("y", [TOK, D], f32, kind="ExternalOutput")
    io["dbg"] = None
    if dbg:
        io["dbg"] = {
            "qT": nc.dram_tensor("dbg_qT", [NP, 128, TOK], bf16, kind="ExternalOutput"),
            "kT": nc.dram_tensor("dbg_kT", [NP, 128, S], bf16, kind="ExternalOutput"),
            "v": nc.dram_tensor("dbg_v", [NKT, 128, D], bf16, kind="ExternalOutput"),
            "A0": nc.dram_tensor("dbg_A0", [128, H, QB], bf16, kind="ExternalOutput"),
            "den0": nc.dram_tensor("dbg_den0", [128, QB], f32, kind="ExternalOutput"),
            "vT": nc.dram_tensor("dbg_vT", [NP, 128, QB], f32, kind="ExternalOutput"),
            "x0": nc.dram_tensor("dbg_x0", [2, 128, D], f32, kind="ExternalOutput"),
            "x1": nc.dram_tensor("dbg_x1", [2, 128, D], f32, kind="ExternalOutput"),
            "h1": nc.dram_tensor("dbg_h1", [4, 128, QB], bf16, kind="ExternalOutput"),
            "z": nc.dram_tensor("dbg_z", [2, 128, D], f32, kind="ExternalOutput"),
        }

    with tile.TileContext(nc) as tc:
        from contextlib import ExitStack
        with ExitStack() as ctx:
            _emit(nc, tc, rep, io, ctx, phases)
    nc.compile()
    return nc


def _bcast_mid(t, n):
    # [128, Q] AP -> [128, n, Q] with a zero-step middle dim (free re-read)
    a = t[:]
    return bass.AP(tensor=a.tensor, offset=a.offset,
                   ap=[a.ap[0], [0, n], a.ap[1]])


def _emit(nc, tc, rep, io, ctx, phases=99):
    const = ctx.enter_context(tc.tile_pool(name="const", bufs=1))
    C = {}
    C["identr"] = const.tile([128, 128], f32r, name="identr_t")
    C["ones1"] = const.tile([1, 128], bf16, name="ones1_t")
    C["bqk"] = const.tile([128, 16], f32, name="bqk_t")
    C["b1p"] = const.tile([128, 32], f32, name="b1p_t")
    C["bvr"] = const.tile([1, D], bf16, name="bvr_t")
    C["b2r"] = const.tile([1, D], bf16, name="b2r_t")
    for nm in ("identr", "ones1", "bqk", "b1p", "bvr", "b2r"):
        nc.sync.dma_start(out=C[nm][:], in_=io[nm][:])
    C["eps"] = const.tile([128, 1], f32, name="eps_t")
    nc.vector.memset(C["eps"], EPS)
    # replicated [128, D] bf16 row constants (DVE cannot partition-broadcast)
    for nm in ("g1", "be1", "g2", "be2"):
        t = const.tile([128, D], bf16, name=f"row_{nm}")
        a = io[nm][:]
        nc.gpsimd.dma_start(out=t[:], in_=bass.AP(
            tensor=a.tensor, offset=a.offset, ap=[[0, 128], a.ap[1]]))
        C[nm] = t

    for r in range(rep):
        from contextlib import ExitStack
        with ExitStack() as bctx:
            _emit_body(nc, tc, r, io, C, bctx, io.get("dbg") if r == 0 else None, phases)


def _emit_body(nc, tc, r, io, C, bctx, dbg=None, phases=99):
    p = f"r{r}_"
    xT, xqT, xs = io["xT"], io["xqT"], io["xs"]
    wq, wk, wv, wo, w1, w2, y = (io["wq"], io["wk"], io["wv"], io["wo"],
                                 io["w1"], io["w2"], io["y"])
    identr, ones1, bvr, b2r = C["identr"], C["ones1"], C["bvr"], C["b2r"]

    kvq = bctx.enter_context(tc.tile_pool(name=p + "kvq", bufs=1))
    kT_sb = [kvq.tile([128, S], bf16, name=f"{p}kT{i}") for i in range(NP)]
    qT_sb = [kvq.tile([128, TOK], bf16, name=f"{p}qT{i}") for i in range(NP)]
    v_sb = [kvq.tile([128, D], bf16, name=f"{p}v{i}") for i in range(NKT)]

    ps_mm = bctx.enter_context(tc.tile_pool(name=p + "ps_mm", bufs=2,
                                            space="PSUM"))
    ps_sc = bctx.enter_context(tc.tile_pool(name=p + "ps_sc", bufs=2,
                                            space="PSUM"))
    ps_av = bctx.enter_context(tc.tile_pool(name=p + "ps_av", bufs=1,
                                            space="PSUM"))
    xst = bctx.enter_context(tc.tile_pool(name=p + "xst", bufs=16))
    att = bctx.enter_context(tc.tile_pool(name=p + "att", bufs=1))
    work = bctx.enter_context(tc.tile_pool(name=p + "work", bufs=1))

    def wtile(nm, dt=f32r):
        return xst.tile([128, 512], dt, tag="xtc", name=nm)

    # ---- K^T projection: kT[pp] = [dk(2 heads x 64), tok] (bf16) ----
    wkh = [xst.tile([128, D], bf16, tag="xtc", name=f"{p}wkh{k}")
           for k in range(8)]
    for k in range(8):
        nc.sync.dma_start(out=wkh[k][:], in_=wk[k * 128:(k + 1) * 128, :])
    for cc in range(2):
        xcb = [xst.tile([128, 1024], bf16, tag="xtc", name=f"{p}xcb{cc}_{k}")
               for k in range(8)]
        for k in range(8):
            nc.sync.dma_start(out=xcb[k][:],
                                in_=xT[k * 128:(k + 1) * 128,
                                      cc * 1024:(cc + 1) * 1024])
        for h5 in range(2):
            c = cc * 2 + h5
            for pp in range(NP):
                ps = ps_mm.tile([128, 512], f32, tag="mm",
                                name=f"{p}kps{c}_{pp}")
                for k in range(8):
                    nc.tensor.matmul(ps[:], wkh[k][:, pp * 128:(pp + 1) * 128],
                                     xcb[k][:, h5 * 512:(h5 + 1) * 512],
                                     start=(k == 0), stop=(k == 7))
                nc.scalar.activation(kT_sb[pp][:, c * 512:(c + 1) * 512],
                                     ps[:], AF.Identity,
                                     bias=C["bqk"][:, 8 + pp:9 + pp],
                                     scale=1.0)

    # ---- Q^T projection (bf16) ----
    wqh = [xst.tile([128, D], bf16, tag="xtc", name=f"{p}wqh{k}")
           for k in range(8)]
    xqb = [xst.tile([128, TOK], bf16, tag="xtc", name=f"{p}xqb{k}")
           for k in range(8)]
    for k in range(8):
        nc.sync.dma_start(out=wqh[k][:], in_=wq[k * 128:(k + 1) * 128, :])
        nc.sync.dma_start(out=xqb[k][:], in_=xqT[k * 128:(k + 1) * 128, :])
    for pp in range(NP):
        ps = ps_mm.tile([128, TOK], f32, tag="mm", name=f"{p}qps{pp}")
        for k in range(8):
            nc.tensor.matmul(ps[:], wqh[k][:, pp * 128:(pp + 1) * 128],
                             xqb[k][:], start=(k == 0), stop=(k == 7))
        nc.scalar.activation(qT_sb[pp][:], ps[:], AF.Identity,
                             bias=C["bqk"][:, pp:pp + 1], scale=1.0)

    # ---- V projection (bf16): emitted per-kt inside attention(0) ----
    wvh = [xst.tile([128, D], bf16, tag="xtc", name=f"{p}wvh{k}")
           for k in range(8)]
    for k in range(8):
        nc.sync.dma_start(out=wvh[k][:], in_=wv[k * 128:(k + 1) * 128, :])
    xvb_cur = [None]

    def v_emit(kt):
        cc, kq = kt // 8, kt % 8
        if kq == 0:
            xvb_cur[0] = [xst.tile([128, 1024], bf16, tag="xtc",
                                   name=f"{p}xvb{cc}_{k}") for k in range(8)]
            for k in range(8):
                nc.sync.dma_start(out=xvb_cur[0][k][:],
                                    in_=xT[k * 128:(k + 1) * 128,
                                          cc * 1024:(cc + 1) * 1024])
        xvb = xvb_cur[0]
        for n in range(2):
            ps = ps_mm.tile([128, 512], f32, tag="mm", name=f"{p}vps{kt}_{n}")
            for k in range(8):
                nc.tensor.matmul(ps[:], xvb[k][:, kq * 128:(kq + 1) * 128],
                                 wvh[k][:, n * 512:(n + 1) * 512],
                                 start=(k == 0), stop=False)
            nc.tensor.matmul(ps[:], ones1[:], bvr[:, n * 512:(n + 1) * 512],
                             start=False, stop=True)
            nc.vector.tensor_copy(v_sb[kt][:, n * 512:(n + 1) * 512], ps[:])

    x1_t = {}     # (qb, t) -> [128, D] f32r (LN1 out)
    x1T = {}      # (qb, j) -> [128, QB] bf16 (transposed LN1 out)
    vT = {}       # (qb, pp) -> [128, QB] f32r

    av = {}
    for qb in range(NQB):
        av[qb] = [ps_av.tile([128, 512], f32, tag=f"av{j}",
                             name=f"{p}av{qb}_{j}") for j in range(4)]

    def attention(qb):
        # multiple col-tiled accumulation streams share an av bank; start=True
        # clears per-bank state and corrupts sibling streams (verified on HW).
        # Pre-zero via DVE and accumulate with start=False throughout.
        for j in range(4):
            nc.vector.memset(av[qb][j][:], 0.0)
        for kt in range(NKT):
            if qb == 0:
                v_emit(kt)
            expall = att.tile([128, H, QB], bf16, tag="expall",
                              name=f"{p}ea{qb}_{kt}", bufs=3)
            for pp in range(NP):
                scs = []
                for i in range(2):
                    sc = ps_sc.tile([128, QB], f32, tag="sc",
                                    name=f"{p}sc{qb}_{kt}_{pp}_{i}")
                    nc.tensor.matmul(
                        sc[:],
                        kT_sb[pp][i * 64:(i + 1) * 64,
                                  kt * 128:(kt + 1) * 128],
                        qT_sb[pp][i * 64:(i + 1) * 64,
                                  qb * QB:(qb + 1) * QB],
                        start=True, stop=True, tile_position=(i * 64, 0))
                    scs.append(sc)
                for i in range(2):
                    nc.scalar.activation(expall[:, 2 * pp + i, :], scs[i][:],
                                         AF.Exp, scale=0.125)
            d4a = att.tile([128, 4, QB], bf16, tag="d4a", bufs=2,
                           name=f"{p}d4a_{qb}_{kt}")
            nc.vector.tensor_add(d4a[:], expall[:, 0:4, :], expall[:, 4:8, :])
            d4b = att.tile([128, 4, QB], bf16, tag="d4b", bufs=2,
                           name=f"{p}d4b_{qb}_{kt}")
            nc.vector.tensor_add(d4b[:], expall[:, 8:12, :],
                                 expall[:, 12:16, :])
            d2 = att.tile([128, 4, QB], bf16, tag="d2", bufs=2,
                          name=f"{p}d2_{qb}_{kt}")
            nc.vector.tensor_add(d2[:], d4a[:], d4b[:])
            d1 = att.tile([128, 2, QB], bf16, tag="d1", bufs=2,
                          name=f"{p}d1_{qb}_{kt}")
            nc.vector.tensor_add(d1[:], d2[:, 0:2, :], d2[:, 2:4, :])
            den = att.tile([128, QB], f32, tag="den", bufs=2,
                           name=f"{p}dn{qb}_{kt}")
            nc.vector.tensor_add(den[:], d1[:, 0, :], d1[:, 1, :])
            rden = att.tile([128, QB], f32, tag="rden", bufs=2,
                            name=f"{p}rd{qb}_{kt}")
            nc.vector.reciprocal_approx_fast(out=rden[:], in_=den[:])
            rdenb = att.tile([128, QB], bf16, tag="rdenb", bufs=2,
                             name=f"{p}rb{qb}_{kt}")
            nc.vector.tensor_copy(rdenb[:], rden[:])
            # normalize in place: A = exp * (1/den), den broadcast over heads
            nc.vector.tensor_mul(expall[:, 0:10, :], expall[:, 0:10, :],
                                 _bcast_mid(rdenb, 10))
            nc.gpsimd.tensor_mul(expall[:, 10:16, :], expall[:, 10:16, :],
                                 _bcast_mid(rdenb, 6))
            if dbg is not None and qb == 0 and kt == 0:
                nc.sync.dma_start(out=dbg["A0"][:], in_=expall[:])
                nc.sync.dma_start(out=dbg["den0"][:], in_=den[:])
            for pp in range(NP):
                j, half = pp // 2, pp % 2
                for i in range(2):
                    hg = 2 * pp + i
                    nc.tensor.matmul(
                        av[qb][j][i * 64:(i + 1) * 64,
                                  half * QB:(half + 1) * QB],
                        v_sb[kt][:, hg * 64:(hg + 1) * 64],
                        expall[:, hg, :],
                        start=False, stop=(kt == NKT - 1),
                        tile_position=(0, i * 64))

    def vals(qb):
        for pp in range(NP):
            j, half = pp // 2, pp % 2
            t = work.tile([128, QB], f32r, tag=f"vT{pp}",
                          name=f"{p}vT{qb}_{pp}")
            nc.vector.tensor_copy(t[:], av[qb][j][:, half * QB:(half + 1) * QB])
            if dbg is not None and qb == 0:
                nc.sync.dma_start(out=dbg["vT"][pp], in_=t[:].bitcast(f32))
            vT[(qb, pp)] = t

    def layernorm(nm, x0, gname, bname, out_t):
        # x0: [128, D] f32 sbuf tile; normalizes in place then writes out_t
        stats = work.tile([128, 2, 6], f32, tag="lnst", name=nm + "_st",
                          bufs=2)
        for s in range(2):
            nc.vector.bn_stats(out=stats[:, s, :],
                               in_=x0[:, s * 512:(s + 1) * 512])
        mv = work.tile([128, 2], f32, tag="lnmv", name=nm + "_mv", bufs=2)
        nc.vector.bn_aggr(out=mv[:], in_=stats[:])
        lnv = work.tile([128, 1], f32, tag="lnv", name=nm + "_lv", bufs=2)
        nc.scalar.activation(lnv[:], mv[:, 1:2], AF.Ln, bias=C["eps"][:],
                             scale=1.0)
        rstd = work.tile([128, 1], f32, tag="lnr", name=nm + "_lr", bufs=2)
        nc.scalar.activation(rstd[:], lnv[:], AF.Exp, scale=-0.5)
        nc.vector.tensor_scalar(x0[:], x0[:], mv[:, 0:1], rstd[:],
                                op0=ALU.subtract, op1=ALU.mult)
        nc.vector.tensor_mul(x0[:], x0[:], C[gname][:])
        nc.vector.tensor_add(out_t[:], x0[:], C[bname][:])

    def proj_ln1(qb):
        x0s = {}
        for n in range(2):
            woc = [wtile(f"{p}wo{qb}_{n}_{k}") for k in range(8)]
            for k in range(8):
                nc.sync.dma_start(out=woc[k][:],
                                  in_=wo[k * 128:(k + 1) * 128,
                                        n * 512:(n + 1) * 512])
            for t in range(2):
                if n == 0:
                    x0s[t] = work.tile([128, D], f32, tag="x0",
                                       name=f"{p}x0_{qb}_{t}", bufs=2)
                row = qb * QB + t * 128
                xst_t = wtile(f"{p}xsl{qb}_{n}_{t}")
                nc.sync.dma_start(out=xst_t[:],
                                  in_=xs[row:row + 128,
                                         n * 512:(n + 1) * 512])
                ps = ps_mm.tile([128, 512], f32, tag="mm",
                                name=f"{p}ops{qb}_{n}_{t}")
                for k in range(8):
                    nc.tensor.matmul(ps[:],
                                     vT[(qb, k)][:, t * 128:(t + 1) * 128],
                                     woc[k][:], start=(k == 0), stop=False)
                nc.tensor.matmul(ps[:], identr[:], xst_t[:],
                                 start=False, stop=True)
                nc.scalar.copy(x0s[t][:, n * 512:(n + 1) * 512], ps[:])
        if dbg is not None and qb == 0:
            for t in range(2):
                nc.sync.dma_start(out=dbg["x0"][t], in_=x0s[t][:])
        for t in range(2):
            x1 = work.tile([128, D], f32r, tag="x1", name=f"{p}x1_{qb}_{t}",
                           bufs=2)
            layernorm(f"{p}ln1_{qb}_{t}", x0s[t], "g1", "be1", x1)
            if dbg is not None and qb == 0:
                nc.sync.dma_start(out=dbg["x1"][t], in_=x1[:].bitcast(f32))
            x1_t[(qb, t)] = x1
            for j in range(8):
                tr = ps_mm.tile([128, 128], f32r, tag="mm",
                                name=f"{p}tr{qb}_{t}_{j}")
                nc.tensor.transpose(tr[:], x1[:, j * 128:(j + 1) * 128],
                                    identr[:])
                if t == 0:
                    x1T[(qb, j)] = work.tile([128, QB], bf16, tag=f"x1T{j}",
                                             name=f"{p}x1T{qb}_{j}")
                nc.vector.tensor_copy(x1T[(qb, j)][:, t * 128:(t + 1) * 128], tr[:])

    def ffn(qb):
        h1 = {}
        for mc in range(4):
            w1c = [xst.tile([128, 1024], bf16, tag="xtc",
                            name=f"{p}w1_{qb}_{mc}_{k}") for k in range(8)]
            for k in range(8):
                nc.sync.dma_start(out=w1c[k][:],
                                  in_=w1[k * 128:(k + 1) * 128,
                                        mc * 1024:(mc + 1) * 1024])
            for mi in range(8):
                m = mc * 8 + mi
                ps = ps_mm.tile([128, QB], f32, tag="mm",
                                name=f"{p}f1ps{qb}_{m}")
                for k in range(8):
                    nc.tensor.matmul(ps[:],
                                     w1c[k][:, mi * 128:(mi + 1) * 128],
                                     x1T[(qb, k)][:],
                                     start=(k == 0), stop=(k == 7))
                ht = work.tile([128, QB], bf16, tag=f"h1_{m}",
                               name=f"{p}h1_{qb}_{m}")
                nc.scalar.activation(ht[:], ps[:], AF.Relu,
                                     bias=C["b1p"][:, m:m + 1], scale=1.0)
                if dbg is not None and qb == 0 and m < 4:
                    nc.sync.dma_start(out=dbg["h1"][m], in_=ht[:])
                h1[m] = ht
        for n in range(2):
            w2c = [xst.tile([128, 1024], bf16, tag="xtc",
                            name=f"{p}w2_{qb}_{n}_{g}") for g in range(16)]
            for g in range(16):
                nc.sync.dma_start(out=w2c[g][:],
                                    in_=w2[n, :, g * 1024:(g + 1) * 1024])
            for t in range(2):
                if n == 0:
                    x0s_f = work.tile([128, D], f32, tag="x0",
                                      name=f"{p}z{qb}_{t}", bufs=2)
                    zs[t] = x0s_f
                ps = ps_mm.tile([128, 512], f32, tag="mm",
                                name=f"{p}f2ps{qb}_{n}_{t}")
                for m in range(32):
                    nc.tensor.matmul(ps[:], h1[m][:, t * 128:(t + 1) * 128],
                                     w2c[m // 2][:, (m % 2) * 512:
                                                 (m % 2 + 1) * 512],
                                     start=(m == 0), stop=False)
                nc.tensor.matmul(ps[:], identr[:],
                                 x1_t[(qb, t)][:, n * 512:(n + 1) * 512],
                                 start=False, stop=False)
                nc.tensor.matmul(ps[:], ones1[:],
                                 b2r[:, n * 512:(n + 1) * 512],
                                 start=False, stop=True)
                nc.scalar.copy(zs[t][:, n * 512:(n + 1) * 512], ps[:])
        if dbg is not None and qb == 0:
            for t in range(2):
                nc.sync.dma_start(out=dbg["z"][t], in_=zs[t][:])
        for t in range(2):
            yrow = qb * QB + t * 128
            layernorm(f"{p}ln2_{qb}_{t}", zs[t], "g2", "be2", zs[t])
            nc.sync.dma_start(out=y[yrow:yrow + 128, :], in_=zs[t][:])

    zs = {}
    steps = [lambda: attention(0), lambda: vals(0), lambda: proj_ln1(0),
             lambda: attention(1), lambda: ffn(0), lambda: vals(1),
             lambda: proj_ln1(1), lambda: ffn(1)]
    for i, s in enumerate(steps):
        if i + 1 <= phases:
            s()


_cached = None


def _get_program():
    global _cached
    if _cached is None:
        _cached = build(rep=1)
    return _cached


def _tile_w2(w2):
    import ml_dtypes
    # [FF, D] -> [2, 128, FF*4]: w2t[n, p, m*512+c] = w2[m*128+p, n*512+c]
    w4 = w2.reshape(32, 128, 2, 512)            # m, p, n, c
    out = np.ascontiguousarray(w4.transpose(2, 1, 0, 3).reshape(2, 128, FF * 4))
    return out.astype(ml_dtypes.bfloat16)


def prepare_inputs(x, w_qkv, b_qkv, w_o, b_o, gamma1, beta1, w1, b1, w2, b2,
                   gamma2, beta2):
    import ml_dtypes
    x = np.asarray(x, np.float32)
    w_qkv = np.asarray(w_qkv, np.float32)
    b_qkv = np.asarray(b_qkv, np.float32)
    wq3 = w_qkv.reshape(D, H, 3 * HD)
    bq3 = b_qkv.reshape(H, 3 * HD)
    bq_h = bq3[:, 0:HD].reshape(D)
    bk_h = bq3[:, HD:2 * HD].reshape(D)
    bqk = np.stack([bq_h.reshape(8, 128), bk_h.reshape(8, 128)],
                   axis=0).reshape(16, 128).T  # [128, 16]: cols 0-7 q, 8-15 k
    shared = {
        "wq": np.asarray(wq3[:, :, 0:HD].reshape(D, D), ml_dtypes.bfloat16),
        "wk": np.asarray(wq3[:, :, HD:2 * HD].reshape(D, D),
                         ml_dtypes.bfloat16),
        "wv": np.asarray(wq3[:, :, 2 * HD:3 * HD].reshape(D, D),
                         ml_dtypes.bfloat16),
        "bqk": np.ascontiguousarray(bqk, dtype=np.float32),
        "bvr": np.asarray(bq3[:, 2 * HD:3 * HD].reshape(1, D),
                          ml_dtypes.bfloat16),
        "wo": np.asarray(w_o, np.float32),
        "w1": np.asarray(w1, ml_dtypes.bfloat16),
        "b1p": np.ascontiguousarray(
            np.asarray(b1, np.float32).reshape(32, 128).T),
        "w2": _tile_w2(np.asarray(w2, np.float32)),
        "b2r": np.asarray(np.asarray(b2, np.float32).reshape(1, D),
                          ml_dtypes.bfloat16),
        "g1": np.asarray(gamma1, np.float32).reshape(1, D),
        "be1": np.asarray(beta1, np.float32).reshape(1, D),
        "g2": np.asarray(gamma2, np.float32).reshape(1, D),
        "be2": np.asarray(beta2, np.float32).reshape(1, D),
        "identr": np.eye(128, dtype=np.float32),
        "ones1": np.ones((1, 128), ml_dtypes.bfloat16),
    }
    bo = np.asarray(b_o, np.float32)
    in_maps = []
    for c in range(NCORES):
        b, q0 = c // 4, (c % 4) * TOK
        xTb = np.ascontiguousarray(x[b].T)
        m = dict(shared)
        m["xT"] = np.asarray(xTb, ml_dtypes.bfloat16)
        m["xqT"] = np.asarray(xTb[:, q0:q0 + TOK], ml_dtypes.bfloat16)
        m["xs"] = np.ascontiguousarray(x[b, q0:q0 + TOK] + bo[None, :])
        in_maps.append(m)
    return in_maps


def kernel(**inputs):
    from concourse.bass_utils import run_bass_kernel_spmd
    nc = _get_program()
    in_maps = prepare_inputs(**inputs)
    res = run_bass_kernel_spmd(nc, in_maps, list(range(NCORES)))
    out = np.empty((2, S, D), np.float32)
    for c in range(NCORES):
        b, q0 = c // 4, (c % 4) * TOK
        out[b, q0:q0 + TOK] = res.results[c]["y"]
    return out
